# revision 2
# baseline (speedup 1.0000x reference)
"""Trainium2 Bass kernel for DfaRnn forward: out[b,t] = tanh(x_t @ W_xh + h_{t-1} @ W_hh + b).

Strategy (8 NeuronCores, data-parallel over batch, B=16 -> 2 rows/core):
  - Host pre-transposes inputs so the device kernel needs no transposes:
      xt[p, dc, t, b]  = x[b, t, dc*128+p]          (fp16)
      wxh[p, dc, mc, j] = W_xh[dc*128+p, mc*128+j]  (fp16)
      whh[p, kc, mc, j] = W_hh[kc*128+p, mc*128+j]  (fp16)
      bt16[0, j]       = b[j]                       (fp16)
  - xp = x @ W_xh + b is matmul'd directly INTO PSUM (a [1,128] b-tile against
    a ones row adds the bias) and STAYS there: PSUM's 8 banks = 4 m-chunks x
    2 step-parity lanes, each bank holding a 256-step window of xp columns.
    The PE refills consumed windows in-line between sweeps (amortized ~17ns
    per step); xproj matmuls set has_written, so the recurrence matmuls
    accumulate straight onto xp with start=False.
  - Sequential phase (T steps): h_t kept as hT [4x128 partitions, 2 cols].
    Per step: 16 matmuls (W_hh tiles stationary, h columns moving) accumulate
    onto the step's xp columns; ScalarE reads z = xp + W@h straight from PSUM
    and writes tanh -> hs fp16 (which doubles as the next step's rhs history
    and the output staging buffer). No VectorE op on the critical path: the
    release chain is PE -> ACT -> PE. Consecutive steps alternate parity
    lanes, so the PE never writes a bank ScalarE is still reading.
    The step is split in halves (tanh over m-chunks 01 / 23; matmul order:
    6 kc01-MMs, wait for tanh h2 of t-1, finish groups 01 at MM 10, rest).
  - Output hs[p, mc, t, b] fp16 DMA'd out; host reassembles [B, T, H] fp32.

Numerics: fp16 storage for W/x/h with fp32 PSUM accumulation. Measured: global
rel err ~4.5e-4 vs the fp32 reference (errors are contractive). Output f32.
"""

import os
import sys

import numpy as np

for _p in ("/opt/trn_rl_repo",):
    if os.path.isdir(_p) and _p not in sys.path:
        sys.path.append(_p)

import concourse.bass as bass
import concourse.mybir as mybir
from concourse import bass_utils

P = 128          # partitions
H = 512          # hidden dim
D = 512          # input dim
NCH = H // P     # 4 h-chunks
NCD = D // P     # 4 d-chunks
N_CORES = 8
WSTEPS = 256     # xp steps per PSUM bank (per parity-lane window)

f16 = mybir.dt.float16
f32 = mybir.dt.float32


def build_nc(T: int, b_local: int, strict_sync: bool = False):
    """Build the per-core Bass program (SPMD; same program on all cores).

    strict_sync retained for API compat; the PE<->ACT sync here is all
    direct, so no checker-only waits are needed.
    """
    assert T % 2 == 0
    lane_steps = T // 2                           # steps per parity lane
    n_win = (lane_steps + WSTEPS - 1) // WSTEPS   # windows per lane

    nc = bass.Bass("TRN2", target_bir_lowering=False, debug=False)

    # DRAM I/O
    xt_d = nc.dram_tensor("xt", [P, NCD, T, b_local], f16, kind="ExternalInput")
    wxh_d = nc.dram_tensor("wxh", [P, NCD, NCH, P], f16, kind="ExternalInput")
    whh_d = nc.dram_tensor("whh", [P, NCH, NCH, P], f16, kind="ExternalInput")
    bt_d = nc.dram_tensor("bt16", [1, H], f16, kind="ExternalInput")
    hs_d = nc.dram_tensor("hs", [P, NCH, T, b_local], f16, kind="ExternalOutput")

    # SBUF
    xt = nc.alloc_sbuf_tensor("xt_sb", [P, NCD, T, b_local], f16)
    wxh = nc.alloc_sbuf_tensor("wxh_sb", [P, NCD, NCH, P], f16)
    whh = nc.alloc_sbuf_tensor("whh_sb", [P, NCH, NCH, P], f16)
    bt = nc.alloc_sbuf_tensor("bt16_sb", [1, H], f16)
    ones = nc.alloc_sbuf_tensor("ones_sb", [1, WSTEPS * b_local], f16)
    hs = nc.alloc_sbuf_tensor("hs_sb", [P, NCH, T, b_local], f16)

    # PSUM: 8 banks of [128, 512] f32; bank(2*mc + par) holds a WSTEPS-step
    # window of xp/z columns for chunk mc on parity lane par.
    ps = nc.alloc_psum_tensor("ps", [P, 8, 512], f32)

    in_sem = nc.alloc_semaphore("in_sem")
    ones_sem = nc.alloc_semaphore("ones_sem")
    pe_sem = nc.alloc_semaphore("pe_sem")
    act_sem = nc.alloc_semaphore("act_sem")
    out_sem = nc.alloc_semaphore("out_sem")

    Tanh = mybir.ActivationFunctionType.Tanh
    pitch_xt = NCD * T * b_local
    pitch_ps = 8 * 512

    def win_count(w):
        return min(WSTEPS, lane_steps - w * WSTEPS)

    def ps_col(t):
        # column offset (elements) of step t inside its bank
        return ((t // 2) % WSTEPS) * b_local

    with nc.Block() as block:

        @block.sync
        def _(sync):
            sync.dma_start(xt.ap(), xt_d.ap()).then_inc(in_sem, 16)
            sync.dma_start(wxh.ap(), wxh_d.ap()).then_inc(in_sem, 16)
            sync.dma_start(whh.ap(), whh_d.ap()).then_inc(in_sem, 16)
            sync.dma_start(bt.ap(), bt_d.ap()).then_inc(in_sem, 16)
            sync.wait_ge(act_sem, 2 * T)
            sync.dma_start(hs_d.ap(), hs.ap()).then_inc(out_sem, 16)
            sync.wait_ge(out_sem, 16)

        @block.tensor
        def _(tensor):

            def fill_group(mc, par, w):
                """xproj+b for bank (2*mc+par), window w (steps par+2k,
                k in [w*WSTEPS, w*WSTEPS+cnt)). start=True clears the bank;
                sets has_written so recurrence MMs can accumulate onto it.
                Returns the last matmul."""
                cnt = win_count(w)
                bank = 2 * mc + par
                out_ap = ps[:, bank, 0:cnt * b_local]
                for dc in range(NCD):
                    rhs = bass.AP(
                        xt,
                        dc * T * b_local + (par + 2 * w * WSTEPS) * b_local,
                        [[pitch_xt, P], [2 * b_local, cnt], [1, b_local]],
                    )
                    tensor.matmul(
                        out_ap, wxh[:, dc, mc, :], rhs,
                        start=(dc == 0), stop=False, skip_group_check=True,
                    )
                return tensor.matmul(
                    out_ap, bt[0:1, mc * P:(mc + 1) * P],
                    ones[0:1, 0:cnt * b_local],
                    start=False, stop=True, skip_group_check=True,
                )

            tensor.wait_ge(in_sem, 64)
            tensor.wait_ge(ones_sem, 1)
            # Initial fill: window 0 of both parity lanes. Order so tanh of
            # step 0 (banks par=0, mc01 then mc23) releases earliest.
            fill_group(0, 0, 0)
            fill_group(1, 0, 0).then_inc(pe_sem, 1)   # pe=1: h1(0) ready
            fill_group(2, 0, 0)
            fill_group(3, 0, 0).then_inc(pe_sem, 1)   # pe=2: h2(0) ready
            if lane_steps > 1:
                for mc in range(NCH):
                    fill_group(mc, 1, 0)

            # Refill schedule: window w of lane par is first needed at step
            # t* = 2*WSTEPS*w + par; its banks were last read by ACT at step
            # t*-2, which is complete once act_sem >= 2*(t*-1) — exactly the
            # second wait of sweep t*-1. So the refill goes after sweep
            # u = t*-1 with no extra waits (an explicit ~4.3us PE bubble per
            # lane-window boundary, ~17ns/step amortized).
            refill_after = {}
            for w in range(1, n_win):
                for par in (0, 1):
                    u = 2 * WSTEPS * w + par - 1
                    refill_after[u] = [(mc, par, w) for mc in range(NCH)]

            # Recurrence sweeps. pe_sem: 2 incs/sweep (+2 from initial fill).
            # act_sem: 2 incs/step (tanh halves).
            for t in range(1, T):
                par = t % 2
                col = ps_col(t)

                def mm(mc, kc):
                    return tensor.matmul(
                        ps[:, 2 * mc + par, col:col + b_local],
                        whh[:, kc, mc, :],
                        hs[:, kc, t - 1, :],
                        start=False, stop=(kc == 3),
                        skip_group_check=True,
                    )

                # order optimized for the steady-state cycle (see docstring)
                tensor.wait_ge(act_sem, 2 * t - 1)
                for mc, kc in ((0, 0), (0, 1), (1, 0), (1, 1), (2, 0), (2, 1)):
                    mm(mc, kc)
                tensor.wait_ge(act_sem, 2 * t)
                for mc, kc in ((0, 2), (0, 3), (1, 2), (1, 3)):
                    m = mm(mc, kc)
                m.then_inc(pe_sem, 1)
                for mc, kc in ((3, 0), (3, 1), (2, 2), (2, 3), (3, 2), (3, 3)):
                    m = mm(mc, kc)
                m.then_inc(pe_sem, 1)

                for (mc, rpar, w) in refill_after.get(t, ()):
                    fill_group(mc, rpar, w)

        @block.vector
        def _(vector):
            # ones row for the bias matmul; nothing else — VectorE is off the
            # critical path entirely in this design.
            vector.memset(ones.ap(), 1.0).then_inc(ones_sem, 1)

        @block.scalar
        def _(scalar):
            for t in range(T):
                par = t % 2
                col = ps_col(t)
                for half in (0, 1):
                    scalar.wait_ge(pe_sem, 2 * t + half + 1)
                    src = bass.AP(
                        ps,
                        (4 * half + par) * 512 + col,
                        [[pitch_ps, P], [2 * 512, 2], [1, b_local]],
                    )
                    scalar.activation(
                        hs[:, 2 * half:2 * half + 2, t, :], src, Tanh,
                    ).then_inc(act_sem, 1)

    return nc


def prep_inputs(x, W_xh, W_hh, b, b_local):
    """Host-side layout transforms. Returns per-core input maps."""
    T = x.shape[1]
    wxh_np = np.ascontiguousarray(
        W_xh.reshape(NCD, P, NCH, P).transpose(1, 0, 2, 3)).astype(np.float16)
    whh_np = np.ascontiguousarray(
        W_hh.reshape(NCH, P, NCH, P).transpose(1, 0, 2, 3)).astype(np.float16)
    bt_np = b.reshape(1, H).astype(np.float16)
    in_maps = []
    for c in range(N_CORES):
        xc = x[c * b_local:(c + 1) * b_local]  # [b_local, T, D]
        xt_np = np.ascontiguousarray(
            xc.transpose(2, 1, 0).reshape(NCD, P, T, b_local).transpose(1, 0, 2, 3)
        ).astype(np.float16)
        in_maps.append({"xt": xt_np, "wxh": wxh_np, "whh": whh_np, "bt16": bt_np})
    return in_maps


def assemble_output(core_outs, T, b_local):
    B = N_CORES * b_local
    full = np.empty((B, T, H), np.float32)
    for c in range(N_CORES):
        hs_np = core_outs[c]["hs"]  # [P, NCH, T, b_local] fp16
        full[c * b_local:(c + 1) * b_local] = (
            hs_np.transpose(3, 2, 1, 0).reshape(b_local, T, H).astype(np.float32))
    return full


_NC_CACHE = {}


def _get_nc(T, b_local):
    key = (T, b_local)
    if key not in _NC_CACHE:
        _NC_CACHE[key] = build_nc(T, b_local)
    return _NC_CACHE[key]


def prepare(inputs):
    """Build (nc, in_maps, assemble_fn) for the given full inputs."""
    x = np.asarray(inputs["x"], np.float32)
    W_xh = np.asarray(inputs["W_xh"], np.float32)
    W_hh = np.asarray(inputs["W_hh"], np.float32)
    b = np.asarray(inputs["b"], np.float32)
    # A affects only the backward pass; the forward output does not use it.
    B, T, D_ = x.shape
    assert D_ == D and W_xh.shape == (D, H) and W_hh.shape == (H, H)
    assert B % N_CORES == 0
    b_local = B // N_CORES

    nc = _get_nc(T, b_local)
    in_maps = prep_inputs(x, W_xh, W_hh, b, b_local)

    def assemble(core_outs):
        return assemble_output(core_outs, T, b_local)

    return nc, in_maps, assemble


def run_on_device(inputs, trace=False, **spmd_kwargs):
    nc, in_maps, assemble = prepare(inputs)
    res = bass_utils.run_bass_kernel_spmd(
        nc, in_maps, core_ids=list(range(N_CORES)), trace=trace, **spmd_kwargs)
    return assemble(res.results), res


def kernel(**inputs):
    try:
        out, _ = run_on_device(inputs)
        return out
    except Exception:
        # One retry: a rare transient NRT/dispatch failure was observed under
        # heavy repeated execution; a fresh attempt (re-lower + re-execute)
        # recovers when the device session is still healthy.
        import time as _time

        _time.sleep(2.0)
        try:
            import jax as _jax

            _jax.clear_caches()
        except Exception:
            pass
        out, _ = run_on_device(inputs)
        return out

